# revision 8
# baseline (speedup 1.0000x reference)
"""Trainium2 Bass kernel for nn_Decimate: 129-tap polyphase FIR decimation by q=4.

The reference's blocked-FFT conv is mathematically a strided valid correlation
    y[b, i] = sum_{j=0}^{128} x_ext[b, 4i + j] * k[j],   i in [0, 262144)
where x_ext = [reflect_64(x), x, zeros_64]  (length 1048704 = 128 * 8193).

Device scheme (per NeuronCore, 2 batch rows each across 8 cores):
  - x_ext is chunked into 512-sample groups; plane_r[col, p] = x_ext[512 col
    + 128 r + p].  Planes are cast to a SINGLE bf16 (the rel-err budget is
    2e-2; bf16-only lands ~3e-3), transposed to partition-major X[p, col]
    and packed per (row, slab) plane-major — all on host, so the device
    does only large plain DMAs (two plane-pair halves per slab).
  - Toeplitz weights W_s[p, i0] = k[128 s + p - 4 i0] (5 shifts), bf16,
    nonzero only on an i0 band: s=0:[0,32) 1:[0,64) 2:[32,96) 3:[64,128)
    4:[96,128).
  - Tensor engine, WEIGHTS stationary / signal moving, one matmul per
    shift covering a whole 512-chunk slab (moving = 512 contiguous plane
    cols), accumulating into one PSUM bank O[i0, c']:
        O[i0, c'] = sum_s W_s[:, i0].T @ X_{s%4}[:, s//4 + c']
    s=1 is loaded full-width (its i0 in [64,128) rows are zeros) and runs
    first with start=True to zero the bank; the rest are banded.
    5 LDWEIGHTS + 5 MATMUL + 1 copy + 1 store per slab.
  - O is copied PSUM->SBUF with a bf16 downcast; stores are contiguous
    1 KiB per partition.  y is produced [row, g, i0, c']; the host
    permutes to chunk-major [row, 512 g + c', i0] and upcasts to fp32
    (output quantization ~1e-3, within budget).
"""

import numpy as np
import ml_dtypes

import concourse.bacc as bacc
import concourse.mybir as mybir
import concourse.tile as tile
from concourse.bass_utils import run_bass_kernel_spmd
from concourse.vector_clock import ScopedClock


class _LeanTile(tile.TileContext):
    """TileContext whose epilogue uses sem-only all-engine barriers.

    Keeps the full shutdown protocol (drain with global-clock waits, barrier,
    semaphore clears, barrier) so NEFF re-execution stays safe, but replaces
    the two drain-based multi_engine_barrier calls with the cheaper
    sem-inc/wait barrier flavor.
    """

    def _drain_and_barrier(self, tick_clock, wait_clock):
        drain_inst = self.nc.sync.drain()
        wait_clock.add_sem_waits(
            drain_inst.ins, ScopedClock({None: tick_clock.global_clock}))
        self.nc.all_engine_barrier(sem_only=True)
        popped = self.nc._tile_sem_poison_stack.pop()
        assert popped is self._sem_poison
        self.nc.clear_and_free_semaphores(
            list(self.sems.allocated().values()))
        self.nc.all_engine_barrier(sem_only=True)


bf16 = ml_dtypes.bfloat16

# Problem constants (hardcoded per harness contract)
T = 1048576
NTAP = 129
Q = 4
PAD = 64
ROWS = 16
N_CORES = 8
ROWS_PER_CORE = ROWS // N_CORES          # 2
OUT = T // Q                             # 262144 outputs per row
CBLK = 128                               # elements per input chunk
NCH_P = 8196                             # chunks, padded to multiple of 4
PLANE_COLS = NCH_P // 4                  # 2049
PLANE_ROWS = 2064                        # padded plane length
NCPRIME = OUT // CBLK                    # 2048 output chunks per row
SLAB_C = 512                             # output-chunk columns per slab
N_SLABS = NCPRIME // SLAB_C              # 4 slab groups per row
PCOLS = 520                              # packed plane cols per slab (513 used)

# All shifts run full-width [0,128) on the output partition dim (the PE
# only allows output base partitions {0,32,64}, and the W planes are zero
# outside their i0 bands anyway, so accumulating the zero rows is free).
# s=1 runs first with start=True to zero the PSUM bank.
# (s, plane, col_off)
COMBO = [(1, 1, 0), (0, 0, 0), (2, 2, 0), (3, 3, 0), (4, 0, 1)]

_PROGRAM = None


def _build_weights(k):
    """W[s, p, i0] = k[128 s + p - 4 i0] masked to j in [0, 128]."""
    W = np.zeros((5, 128, 128), dtype=np.float32)
    p = np.arange(128)[:, None]
    i0 = np.arange(128)[None, :]
    for s in range(5):
        j = 128 * s + p - 4 * i0
        m = (j >= 0) & (j <= 128)
        W[s][m] = k[j[m]]
    return W


def _build_planes(x):
    """x: [B, T] fp32 -> phase planes [B, 4, PLANE_ROWS, 128] fp32."""
    B = x.shape[0]
    xe = np.zeros((B, NCH_P * CBLK), dtype=np.float32)
    xe[:, PAD:PAD + T] = x
    xe[:, :PAD] = x[:, 1:PAD + 1][:, ::-1]
    ch = xe.reshape(B, PLANE_COLS, 4, CBLK)
    planes = np.zeros((B, 4, PLANE_ROWS, CBLK), dtype=np.float32)
    planes[:, :, :PLANE_COLS, :] = ch.transpose(0, 2, 1, 3)
    return planes


def _build_program():
    """Build the per-core Bass/Tile program (same NEFF on all 8 cores)."""
    # Bacc (not raw Bass): its compile() splits multi-wait sync lists into
    # InstEventSemaphore chains — TRN2 allows only 1 wait per instruction.
    nc = bacc.Bacc(None)
    b16 = mybir.dt.bfloat16
    f32 = mybir.dt.float32

    # xs[row, slab, p, plane, c] — per-partition contiguous 4160 B
    xs = nc.declare_dram_parameter(
        "xs", [ROWS_PER_CORE, N_SLABS, CBLK, 4, PCOLS], b16, isOutput=False)
    # w[p, s, i0]
    w = nc.declare_dram_parameter("w", [CBLK, 5, CBLK], b16, isOutput=False)
    # y[row, g, i0, c']: each partition's store is one contiguous 1 KiB
    # burst; the host permutes to chunk-major afterwards.
    y = nc.declare_dram_parameter(
        "y", [ROWS_PER_CORE, N_SLABS, CBLK, SLAB_C], b16, isOutput=True)

    with _LeanTile(nc) as tc:
        with (
            tc.tile_pool(name="wpool", bufs=1) as wpool,
            tc.tile_pool(name="xpool", bufs=8) as xpool,
            tc.tile_pool(name="opool", bufs=8) as opool,
            tc.tile_pool(name="psum", bufs=8, space="PSUM") as psum_pool,
        ):
            w_t = wpool.tile([CBLK, 5, CBLK], b16, tag="w")
            nc.scalar.dma_start(out=w_t[:], in_=w[:])

            for row in range(ROWS_PER_CORE):
                for g in range(N_SLABS):
                    t = xpool.tile([CBLK, 4, PCOLS], b16, tag="xs")
                    # plane-pair halves; shifts s=1,0 only need planes 0-1
                    nc.sync.dma_start(
                        out=t[:, :2, :], in_=xs[row, g, :, :2, :])
                    nc.gpsimd.dma_start(
                        out=t[:, 2:, :], in_=xs[row, g, :, 2:, :])
                    O = psum_pool.tile([CBLK, SLAB_C], f32, tag="O")
                    for i, (s, r, off) in enumerate(COMBO):
                        nc.tensor.matmul(
                            O[:],
                            w_t[:, s, :],
                            t[:, r, off:off + SLAB_C],
                            start=(i == 0), stop=(i == len(COMBO) - 1))
                    stage = opool.tile([CBLK, SLAB_C], b16, tag="stage")
                    nc.vector.tensor_copy(stage[:], O[:])
                    nc.scalar.dma_start(out=y[row, g], in_=stage[:])
    nc.finalize()
    return nc


def _get_program():
    global _PROGRAM
    if _PROGRAM is None:
        _PROGRAM = _build_program()
    return _PROGRAM


def _prepare_in_maps(x, k):
    planes = _build_planes(np.ascontiguousarray(x, dtype=np.float32))
    ph = planes.astype(bf16)
    # host-side transpose to partition-major [B, 4, p, col]
    ph = np.ascontiguousarray(ph.swapaxes(2, 3))

    # pack [B, slab, p, plane, c_local]
    B = x.shape[0]
    xsv = np.zeros((B, N_SLABS, CBLK, 4, PCOLS), dtype=bf16)
    for g in range(N_SLABS):
        c0 = SLAB_C * g
        xsv[:, g, :, :, :] = ph[:, :, :, c0:c0 + PCOLS].swapaxes(1, 2)

    W = _build_weights(np.asarray(k, dtype=np.float32))
    # weight layout [p, s, i0]
    w_t = np.ascontiguousarray(np.transpose(W, (1, 0, 2))).astype(bf16)

    in_maps = []
    for c in range(N_CORES):
        sl = slice(c * ROWS_PER_CORE, (c + 1) * ROWS_PER_CORE)
        in_maps.append({
            "xs": np.ascontiguousarray(xsv[sl]),
            "w": w_t,
        })
    return in_maps


def _run(x, k, trace=False):
    nc = _get_program()
    in_maps = _prepare_in_maps(x, k)
    res = run_bass_kernel_spmd(nc, in_maps, list(range(N_CORES)), trace=trace)
    # device y is [row, g, i0, c']; chunk index = 512 g + c', position = i0
    outs = [
        np.asarray(r["y"]).transpose(0, 1, 3, 2).astype(np.float32)
        for r in res.results
    ]
    out = np.concatenate(outs, axis=0).reshape(ROWS, OUT)
    return out, res


def kernel(x, kernel, q):
    assert int(q) == Q and x.shape == (ROWS, T) and kernel.shape == (NTAP,)
    out, _ = _run(np.asarray(x), np.asarray(kernel), trace=False)
    return out


def kernel_traced(x, kernel, q):
    """Like kernel() but returns (out, BassKernelResults) with HW profile."""
    out, res = _run(np.asarray(x), np.asarray(kernel), trace=True)
    return out, res


# revision 10
# speedup vs baseline: 1.0732x; 1.0732x over previous
"""Trainium2 Bass kernel for nn_Decimate: 129-tap polyphase FIR decimation by q=4.

The reference's blocked-FFT conv is mathematically a strided valid correlation
    y[b, i] = sum_{j=0}^{128} x_ext[b, 4i + j] * k[j],   i in [0, 262144)
where x_ext = [reflect_64(x), x, zeros_64]  (length 1048704 = 128 * 8193).

Device scheme (per NeuronCore, 2 batch rows each across 8 cores):
  - x_ext is chunked into 512-sample groups; plane_r[col, p] = x_ext[512 col
    + 128 r + p].  Planes are cast to a SINGLE bf16 (the rel-err budget is
    2e-2; bf16-only lands ~3e-3), transposed to partition-major X[p, col]
    and packed per (row, slab) plane-major — all on host, so the device
    does only large plain DMAs (two plane-pair halves per slab).
  - Toeplitz weights W_s[p, i0] = k[128 s + p - 4 i0] (5 shifts), bf16,
    nonzero only on an i0 band: s=0:[0,32) 1:[0,64) 2:[32,96) 3:[64,128)
    4:[96,128).
  - Tensor engine, WEIGHTS stationary / signal moving, one matmul per
    shift covering a whole 512-chunk slab (moving = 512 contiguous plane
    cols), accumulating into one PSUM bank O[i0, c']:
        O[i0, c'] = sum_s W_s[:, i0].T @ X_{s%4}[:, s//4 + c']
    s=1 is loaded full-width (its i0 in [64,128) rows are zeros) and runs
    first with start=True to zero the bank; the rest are banded.
    5 LDWEIGHTS + 5 MATMUL + 1 copy + 1 store per slab.
  - O is copied PSUM->SBUF with a bf16 downcast; stores are contiguous
    1 KiB per partition.  y is produced [row, g, i0, c']; the host
    permutes to chunk-major [row, 512 g + c', i0] and upcasts to fp32
    (output quantization ~1e-3, within budget).
"""

import numpy as np
import ml_dtypes

import concourse.bacc as bacc
import concourse.mybir as mybir
import concourse.tile as tile
from concourse.bass_utils import run_bass_kernel_spmd
from concourse.vector_clock import ScopedClock


class _LeanTile(tile.TileContext):
    """TileContext whose epilogue uses sem-only all-engine barriers.

    Keeps the full shutdown protocol (drain with global-clock waits, barrier,
    semaphore clears, barrier) so NEFF re-execution stays safe, but replaces
    the two drain-based multi_engine_barrier calls with the cheaper
    sem-inc/wait barrier flavor.
    """

    def _drain_and_barrier(self, tick_clock, wait_clock):
        drain_inst = self.nc.sync.drain()
        wait_clock.add_sem_waits(
            drain_inst.ins, ScopedClock({None: tick_clock.global_clock}))
        self.nc.all_engine_barrier(sem_only=True)
        popped = self.nc._tile_sem_poison_stack.pop()
        assert popped is self._sem_poison
        self.nc.clear_and_free_semaphores(
            list(self.sems.allocated().values()))
        self.nc.all_engine_barrier(sem_only=True)


bf16 = ml_dtypes.bfloat16

# Problem constants (hardcoded per harness contract)
T = 1048576
NTAP = 129
Q = 4
PAD = 64
ROWS = 16
N_CORES = 8
ROWS_PER_CORE = ROWS // N_CORES          # 2
OUT = T // Q                             # 262144 outputs per row
CBLK = 128                               # elements per input chunk
NCH_P = 8196                             # chunks, padded to multiple of 4
PLANE_COLS = NCH_P // 4                  # 2049
PLANE_ROWS = 2064                        # padded plane length
NCPRIME = OUT // CBLK                    # 2048 output chunks per row
SLAB_C = 512                             # output-chunk columns per slab
N_SLABS = NCPRIME // SLAB_C              # 4 slab groups per row
PCOLS = 516                              # packed plane cols per slab (513 used)
N_WARM = 16                              # PE warm-up matmuls (HAM cold clock)

# All shifts run full-width [0,128) on the output partition dim (the PE
# only allows output base partitions {0,32,64}, and the W planes are zero
# outside their i0 bands anyway, so accumulating the zero rows is free).
# s=1 runs first with start=True to zero the PSUM bank.
# (s, plane, col_off)
COMBO = [(1, 1, 0), (0, 0, 0), (2, 2, 0), (3, 3, 0), (4, 0, 1)]

_PROGRAM = None


def _build_weights(k):
    """W[s, p, i0] = k[128 s + p - 4 i0] masked to j in [0, 128]."""
    W = np.zeros((5, 128, 128), dtype=np.float32)
    p = np.arange(128)[:, None]
    i0 = np.arange(128)[None, :]
    for s in range(5):
        j = 128 * s + p - 4 * i0
        m = (j >= 0) & (j <= 128)
        W[s][m] = k[j[m]]
    return W


def _build_planes(x):
    """x: [B, T] fp32 -> phase planes [B, 4, PLANE_ROWS, 128] fp32."""
    B = x.shape[0]
    xe = np.zeros((B, NCH_P * CBLK), dtype=np.float32)
    xe[:, PAD:PAD + T] = x
    xe[:, :PAD] = x[:, 1:PAD + 1][:, ::-1]
    ch = xe.reshape(B, PLANE_COLS, 4, CBLK)
    planes = np.zeros((B, 4, PLANE_ROWS, CBLK), dtype=np.float32)
    planes[:, :, :PLANE_COLS, :] = ch.transpose(0, 2, 1, 3)
    return planes


def _build_program():
    """Build the per-core Bass/Tile program (same NEFF on all 8 cores)."""
    # Bacc (not raw Bass): its compile() splits multi-wait sync lists into
    # InstEventSemaphore chains — TRN2 allows only 1 wait per instruction.
    nc = bacc.Bacc(None)
    b16 = mybir.dt.bfloat16
    f32 = mybir.dt.float32

    # xs[row, slab, p, plane, c] — per-partition contiguous 4160 B
    xs = nc.declare_dram_parameter(
        "xs", [ROWS_PER_CORE, N_SLABS, CBLK, 4, PCOLS], b16, isOutput=False)
    # w[p, s, i0]
    w = nc.declare_dram_parameter("w", [CBLK, 5, CBLK], b16, isOutput=False)
    # y[row, g, i0, c']: each partition's store is one contiguous 1 KiB
    # burst; the host permutes to chunk-major afterwards.
    y = nc.declare_dram_parameter(
        "y", [ROWS_PER_CORE, N_SLABS, CBLK, SLAB_C], b16, isOutput=True)

    with _LeanTile(nc) as tc:
        with (
            tc.tile_pool(name="wpool", bufs=1) as wpool,
            tc.tile_pool(name="xpool", bufs=8) as xpool,
            tc.tile_pool(name="opool", bufs=8) as opool,
            tc.tile_pool(name="psum", bufs=8, space="PSUM") as psum_pool,
        ):
            w_t = wpool.tile([CBLK, 5, CBLK], b16, tag="w")
            nc.scalar.dma_start(out=w_t[:], in_=w[:])

            # PE warm-up: the HAM clock gate runs the PE at 1.2 GHz until
            # it has seen ~3.4 us of sustained activity.  Burn dummy
            # matmuls on a zeroed tile during the otherwise-dead window
            # between the NRT preamble and the first x slab's arrival so
            # the real matmuls run at 2.4 GHz from the start.
            warm_sb = wpool.tile([CBLK, CBLK], b16, tag="warm")
            nc.gpsimd.memset(warm_sb[:], 0)
            warm_ps = psum_pool.tile([CBLK, SLAB_C], f32, tag="O")
            for i in range(N_WARM):
                nc.tensor.matmul(
                    warm_ps[:, (i % 4) * CBLK:(i % 4 + 1) * CBLK],
                    warm_sb[:], warm_sb[:], start=True, stop=True)

            for row in range(ROWS_PER_CORE):
                for g in range(N_SLABS):
                    t = xpool.tile([CBLK, 4, PCOLS], b16, tag="xs")
                    # one DMA per slab, all on the sync queue: arrival
                    # order matches the in-order PE's consumption order
                    nc.sync.dma_start(out=t[:], in_=xs[row, g])
                    O = psum_pool.tile([CBLK, SLAB_C], f32, tag="O")
                    for i, (s, r, off) in enumerate(COMBO):
                        nc.tensor.matmul(
                            O[:],
                            w_t[:, s, :],
                            t[:, r, off:off + SLAB_C],
                            start=(i == 0), stop=(i == len(COMBO) - 1))
                    stage = opool.tile([CBLK, SLAB_C], b16, tag="stage")
                    nc.vector.tensor_copy(stage[:], O[:])
                    nc.scalar.dma_start(out=y[row, g], in_=stage[:])
    nc.finalize()
    return nc


def _get_program():
    global _PROGRAM
    if _PROGRAM is None:
        _PROGRAM = _build_program()
    return _PROGRAM


def _prepare_in_maps(x, k):
    planes = _build_planes(np.ascontiguousarray(x, dtype=np.float32))
    ph = planes.astype(bf16)
    # host-side transpose to partition-major [B, 4, p, col]
    ph = np.ascontiguousarray(ph.swapaxes(2, 3))

    # pack [B, slab, p, plane, c_local]
    B = x.shape[0]
    xsv = np.zeros((B, N_SLABS, CBLK, 4, PCOLS), dtype=bf16)
    for g in range(N_SLABS):
        c0 = SLAB_C * g
        xsv[:, g, :, :, :] = ph[:, :, :, c0:c0 + PCOLS].swapaxes(1, 2)

    W = _build_weights(np.asarray(k, dtype=np.float32))
    # weight layout [p, s, i0]
    w_t = np.ascontiguousarray(np.transpose(W, (1, 0, 2))).astype(bf16)

    in_maps = []
    for c in range(N_CORES):
        sl = slice(c * ROWS_PER_CORE, (c + 1) * ROWS_PER_CORE)
        in_maps.append({
            "xs": np.ascontiguousarray(xsv[sl]),
            "w": w_t,
        })
    return in_maps


def _run(x, k, trace=False):
    nc = _get_program()
    in_maps = _prepare_in_maps(x, k)
    res = run_bass_kernel_spmd(nc, in_maps, list(range(N_CORES)), trace=trace)
    # device y is [row, g, i0, c']; chunk index = 512 g + c', position = i0
    outs = [
        np.asarray(r["y"]).transpose(0, 1, 3, 2).astype(np.float32)
        for r in res.results
    ]
    out = np.concatenate(outs, axis=0).reshape(ROWS, OUT)
    return out, res


def kernel(x, kernel, q):
    assert int(q) == Q and x.shape == (ROWS, T) and kernel.shape == (NTAP,)
    out, _ = _run(np.asarray(x), np.asarray(kernel), trace=False)
    return out


def kernel_traced(x, kernel, q):
    """Like kernel() but returns (out, BassKernelResults) with HW profile."""
    out, res = _run(np.asarray(x), np.asarray(kernel), trace=True)
    return out, res
